# revision 8
# baseline (speedup 1.0000x reference)
"""Causal self-attention (B=2, T=2048, C=2048, 16 heads) on 8 Trainium2 cores.

Sharding: tensor-parallel over heads — 2 heads per core. Each core computes
q/k/v projections for its head group, causal attention, and a partial output
projection (row-parallel Wo); the host sums the 8 partial outputs.

Device layout notes (per core):
  - All matmuls run in fp32r (full PE rate at moving-dim >= 256).
  - Projections produce qT/kT in [head_dim, token] layout and v in
    [token, head_dim] layout so that attention needs no on-device transposes:
      S^T tile  = kT_tile.T @ qT_window        (matmul lhsT=kT, rhs=qT)
      P^T       = exp(S^T)  (causal-masked via affine_select; no row-max
                  needed: |S| < ~5 for this distribution)
      outT     += v_tile.T @ P^T               (matmul lhsT=v,  rhs=P^T)
      rowsum   += ones.T @ P^T                 (matmul lhsT=ones[128,1])
    softmax normalization is folded in afterwards: outT *= bcast(1/rowsum),
    with the broadcast done by a K=1 matmul of ones[1,128].T @ recip[1,q].
  - 1/sqrt(head_dim) is folded into Wq on the host.
"""

import math
import sys
from contextlib import ExitStack

import numpy as np

sys.path.insert(0, "/opt/trn_rl_repo")

import concourse.bass as bass  # noqa: E402
import concourse.tile as tile  # noqa: E402
from concourse import bacc, mybir  # noqa: E402

F32 = mybir.dt.float32
F32R = mybir.dt.float32r

# Full problem constants
B_FULL, T_FULL, C_FULL = 2, 2048, 2048
N_HEADS, HEAD_DIM = 16, 128
N_CORES = 8
H_LOC = N_HEADS // N_CORES  # 2 heads per core
C_LOC = H_LOC * HEAD_DIM  # 256 output dims per core

WIN = 512  # token window for projections / attention q-window


def build_program(Bb=B_FULL, Tt=T_FULL, Cc=C_FULL):
    """Build the single-core program (SPMD across the 8 cores).

    Per-core DRAM interface:
      xT : [Cc, Bb*Tt]  f32  (x transposed, replicated)
      wq : [Cc, C_LOC]  f32  (Wq rows for this core's heads, transposed,
                              pre-scaled by 1/sqrt(HEAD_DIM))
      wk : [Cc, C_LOC]  f32
      wv : [Cc, C_LOC]  f32
      wo : [C_LOC, Cc]  f32  (Wo columns for this core's heads, transposed)
      y  : [Bb*Tt, Cc]  f32  out (partial sum; host reduces over cores)
    """
    BT = Bb * Tt
    n_kc = Cc // 128  # contraction chunks for projections
    n_win = BT // WIN  # projection token windows
    n_qw = Tt // WIN  # attention q-windows per batch element
    n_bt = BT // 128  # 128-token tiles
    sub = WIN // 128  # 128-token subtiles per window (4)

    nc = bacc.Bacc("TRN2", target_bir_lowering=False, debug=False,
                   num_devices=N_CORES)

    xT_ap = nc.dram_tensor("xT", [Cc, BT], F32R, kind="ExternalInput").ap()
    wq_ap = nc.dram_tensor("wq", [Cc, C_LOC], F32R, kind="ExternalInput").ap()
    wk_ap = nc.dram_tensor("wk", [Cc, C_LOC], F32R, kind="ExternalInput").ap()
    wv_ap = nc.dram_tensor("wv", [Cc, C_LOC], F32R, kind="ExternalInput").ap()
    wo_ap = nc.dram_tensor("wo", [C_LOC, Cc], F32R, kind="ExternalInput").ap()
    y_ap = nc.dram_tensor("y", [BT, Cc], F32, kind="ExternalOutput").ap()

    with tile.TileContext(nc) as tc, ExitStack() as ctx:
        const = ctx.enter_context(tc.tile_pool(name="const", bufs=1))
        wop = ctx.enter_context(tc.tile_pool(name="wop", bufs=1))
        qkv = ctx.enter_context(tc.tile_pool(name="qkv", bufs=1))

        # memset rejects f32r destinations in walrus codegen: set an f32
        # staging tile and convert-copy (bitwise identity) into the f32r one.
        ones_f32 = const.tile([128, 1], F32, tag="ones_f32")
        nc.any.memset(ones_f32[:], 1.0)
        ones_col = const.tile([128, 1], F32R, tag="ones_col")
        nc.vector.tensor_copy(ones_col[:], ones_f32[:])
        ones_rf32 = const.tile([1, 128], F32, tag="ones_rf32")
        nc.any.memset(ones_rf32[:], 1.0)
        ones_row = const.tile([1, 128], F32R, tag="ones_row")
        nc.vector.tensor_copy(ones_row[:], ones_rf32[:])

        # Persistent SBUF tensors
        wo_s = wop.tile([128, H_LOC, Cc], F32R, tag="wo")
        qT_s = qkv.tile([128, H_LOC, BT], F32R, tag="qT")
        kT_s = qkv.tile([128, H_LOC, BT], F32R, tag="kT")
        v_s = qkv.tile([128, n_bt, C_LOC], F32R, tag="v")

        nc.sync.dma_start(wo_s[:], wo_ap.rearrange("(hc p) o -> p hc o", p=128))

        # ---- Stage 1: q/k/v projections --------------------------------
        with nc.named_scope("qkv_proj"), ExitStack() as s1:
            wqkv = s1.enter_context(tc.tile_pool(name="wqkv", bufs=1))
            xpool = s1.enter_context(tc.tile_pool(name="xpool", bufs=3))
            ps_qk = s1.enter_context(
                tc.tile_pool(name="ps_qk", bufs=1, space="PSUM"))
            ps_v = s1.enter_context(
                tc.tile_pool(name="ps_v", bufs=1, space="PSUM"))

            wq_s = wqkv.tile([128, n_kc, C_LOC], F32R, tag="wq")
            wk_s = wqkv.tile([128, n_kc, C_LOC], F32R, tag="wk")
            wv_s = wqkv.tile([128, n_kc, C_LOC], F32R, tag="wv")
            nc.sync.dma_start(wq_s[:],
                              wq_ap.rearrange("(kc p) o -> p kc o", p=128))
            nc.sync.dma_start(wk_s[:],
                              wk_ap.rearrange("(kc p) o -> p kc o", p=128))
            nc.sync.dma_start(wv_s[:],
                              wv_ap.rearrange("(kc p) o -> p kc o", p=128))

            for w in range(n_win):
                toks = slice(w * WIN, (w + 1) * WIN)
                q_ps = [ps_qk.tile([128, WIN], F32, tag=f"q{h}", name=f"q_ps{h}")
                        for h in range(H_LOC)]
                k_ps = [ps_qk.tile([128, WIN], F32, tag=f"k{h}", name=f"k_ps{h}")
                        for h in range(H_LOC)]
                v_ps = [ps_v.tile([128, C_LOC], F32, tag=f"v{j}", name=f"v_ps{j}")
                        for j in range(sub)]
                for kc in range(n_kc):
                    strip = xpool.tile([128, WIN], F32R, tag="strip")
                    nc.sync.dma_start(strip[:],
                                      xT_ap[kc * 128:(kc + 1) * 128, toks])
                    st = (kc == 0)
                    sp = (kc == n_kc - 1)
                    for h in range(H_LOC):
                        hs = slice(h * 128, (h + 1) * 128)
                        nc.tensor.matmul(q_ps[h][:], wq_s[:, kc, hs], strip[:],
                                         start=st, stop=sp)
                        nc.tensor.matmul(k_ps[h][:], wk_s[:, kc, hs], strip[:],
                                         start=st, stop=sp)
                    for j in range(sub):
                        nc.tensor.matmul(v_ps[j][:],
                                         strip[:, j * 128:(j + 1) * 128],
                                         wv_s[:, kc, :], start=st, stop=sp)
                for h in range(H_LOC):
                    nc.scalar.copy(qT_s[:, h, toks], q_ps[h][:])
                    nc.scalar.copy(kT_s[:, h, toks], k_ps[h][:])
                for j in range(sub):
                    nc.vector.tensor_copy(v_s[:, w * sub + j, :], v_ps[j][:])

        # ---- Stage 2: causal attention (+ fused softmax norm) ----------
        with nc.named_scope("attention"), ExitStack() as s2:
            ptpool = s2.enter_context(tc.tile_pool(name="ptpool", bufs=3))
            spool = s2.enter_context(tc.tile_pool(name="spool", bufs=2))
            ps_st = s2.enter_context(
                tc.tile_pool(name="ps_st", bufs=2, space="PSUM"))
            ps_ot = s2.enter_context(
                tc.tile_pool(name="ps_ot", bufs=2, space="PSUM"))
            ps_sum = s2.enter_context(
                tc.tile_pool(name="ps_sum", bufs=2, space="PSUM"))

            # attention output (outT layout) reuses qT_s storage: each
            # qT_s[:, h, qsl] slice is last read by this (b,h,qw)'s S^T
            # matmuls, exactly before ot_s[:, h, qsl] is written.
            ot_s = qT_s

            for b in range(Bb):
                for h in range(H_LOC):
                    for qw in range(n_qw):
                        qoff = b * Tt + qw * WIN
                        qsl = slice(qoff, qoff + WIN)
                        n_kt = sub * (qw + 1)
                        ot_ps = ps_ot.tile([128, WIN], F32, tag="ot")
                        s_ps = ps_sum.tile([1, WIN], F32, tag="s")
                        for kt in range(n_kt):
                            koff = b * Tt + kt * 128
                            st_ps = ps_st.tile([128, WIN], F32, tag="st")
                            nc.tensor.matmul(st_ps[:],
                                             kT_s[:, h, koff:koff + 128],
                                             qT_s[:, h, qsl],
                                             start=True, stop=True)
                            pt = ptpool.tile([128, WIN], F32R, tag="pt")
                            nc.scalar.activation(
                                pt[:], st_ps[:],
                                mybir.ActivationFunctionType.Exp)
                            if kt >= qw * sub:
                                # diagonal-overlap tile: zero entries where
                                # global_k > global_q. Predicate keeps where
                                # base - p + f >= 0, i.e. p - f <= qw*WIN-kt*128
                                # (walrus only implements is_ge/is_gt here).
                                base = qw * WIN - kt * 128
                                nc.gpsimd.affine_select(
                                    out=pt[:], in_=pt[:],
                                    compare_op=mybir.AluOpType.is_ge,
                                    fill=0.0, base=base,
                                    pattern=[[1, WIN]], channel_multiplier=-1,
                                )
                            first = (kt == 0)
                            last = (kt == n_kt - 1)
                            vt = b * (Tt // 128) + kt
                            nc.tensor.matmul(ot_ps[:],
                                             v_s[:, vt, h * 128:(h + 1) * 128],
                                             pt[:], start=first, stop=last)
                            nc.tensor.matmul(s_ps[:], ones_col[:], pt[:],
                                             start=first, stop=last)
                        # normalization: ot_s = ot_ps * bcast(1/s)
                        srec = spool.tile([1, WIN], F32R, tag="srec")
                        with nc.allow_low_precision(reason="f32r == f32 bits"):
                            nc.vector.reciprocal(srec[:], s_ps[:])
                        bc_ps = ps_st.tile([128, WIN], F32, tag="bc")
                        nc.tensor.matmul(bc_ps[:], ones_row[:], srec[:],
                                         start=True, stop=True)
                        nc.scalar.copy(ot_s[:, h, qsl], ot_ps[:])
                        nc.vector.tensor_mul(ot_s[:, h, qsl], ot_s[:, h, qsl],
                                             bc_ps[:])

        # ---- Stage 3: output projection (partial y) --------------------
        with nc.named_scope("out_proj"), ExitStack() as s3:
            ypool = s3.enter_context(tc.tile_pool(name="ypool", bufs=4))
            ps_y = s3.enter_context(
                tc.tile_pool(name="ps_y", bufs=4, space="PSUM"))
            n_nw = Cc // WIN
            for bt in range(n_bt):
                rows = slice(bt * 128, (bt + 1) * 128)
                for nw in range(n_nw):
                    cols = slice(nw * WIN, (nw + 1) * WIN)
                    y_ps = ps_y.tile([128, WIN], F32, tag="y")
                    for hc in range(H_LOC):
                        nc.tensor.matmul(y_ps[:], ot_s[:, hc, rows],
                                         wo_s[:, hc, cols],
                                         start=(hc == 0),
                                         stop=(hc == H_LOC - 1))
                    y_sb = ypool.tile([128, WIN], F32, tag="ysb")
                    nc.vector.tensor_copy(y_sb[:], y_ps[:])
                    nc.sync.dma_start(y_ap[rows, cols], y_sb[:])

    nc.compile()
    return nc


_PROGRAM = None


def _get_program():
    global _PROGRAM
    if _PROGRAM is None:
        _PROGRAM = build_program()
    return _PROGRAM


def make_in_maps(x, Wq, Wk, Wv, Wo):
    """Host-side sharding: build the per-core input dicts."""
    x = np.asarray(x, dtype=np.float32)
    Wq = np.asarray(Wq, dtype=np.float32)
    Wk = np.asarray(Wk, dtype=np.float32)
    Wv = np.asarray(Wv, dtype=np.float32)
    Wo = np.asarray(Wo, dtype=np.float32)
    BT = x.shape[0] * x.shape[1]
    xT = np.ascontiguousarray(x.reshape(BT, -1).T)
    scale = 1.0 / math.sqrt(HEAD_DIM)
    in_maps = []
    for c in range(N_CORES):
        rows = slice(c * C_LOC, (c + 1) * C_LOC)
        in_maps.append({
            "xT": xT,
            "wq": np.ascontiguousarray(Wq[rows, :].T) * scale,
            "wk": np.ascontiguousarray(Wk[rows, :].T),
            "wv": np.ascontiguousarray(Wv[rows, :].T),
            "wo": np.ascontiguousarray(Wo[:, rows].T),
        })
    return in_maps


def kernel(x, Wq, Wk, Wv, Wo):
    from concourse.bass_utils import run_bass_kernel_spmd

    nc = _get_program()
    in_maps = make_in_maps(x, Wq, Wk, Wv, Wo)
    res = run_bass_kernel_spmd(nc, in_maps, list(range(N_CORES)))
    x = np.asarray(x)
    Bb, Tt, Cc = x.shape
    y = np.zeros((Bb * Tt, Cc), dtype=np.float32)
    for c in range(N_CORES):
        y += res.results[c]["y"]
    return y.reshape(Bb, Tt, Cc)


# revision 9
# speedup vs baseline: 1.0623x; 1.0623x over previous
"""Causal self-attention (B=2, T=2048, C=2048, 16 heads) on 8 Trainium2 cores.

Sharding: tensor-parallel over heads — 2 heads per core. Each core computes
q/k/v projections for its head group, causal attention, and a partial output
projection (row-parallel Wo); the host sums the 8 partial outputs.

Device layout notes (per core):
  - All matmuls run in fp32r (full PE rate at moving-dim >= 256).
  - Projections produce qT/kT in [head_dim, token] layout and v in
    [token, head_dim] layout so that attention needs no on-device transposes:
      S^T tile  = kT_tile.T @ qT_window        (matmul lhsT=kT, rhs=qT)
      P^T       = exp(S^T)  (causal-masked via affine_select; no row-max
                  needed: |S| < ~5 for this distribution)
      outT     += v_tile.T @ P^T               (matmul lhsT=v,  rhs=P^T)
      rowsum   += ones.T @ P^T                 (matmul lhsT=ones[128,1])
    softmax normalization is folded in afterwards: outT *= bcast(1/rowsum),
    with the broadcast done by a K=1 matmul of ones[1,128].T @ recip[1,q].
  - 1/sqrt(head_dim) is folded into Wq on the host.
"""

import math
import sys
from contextlib import ExitStack

import numpy as np

sys.path.insert(0, "/opt/trn_rl_repo")

import concourse.bass as bass  # noqa: E402
import concourse.tile as tile  # noqa: E402
from concourse import bacc, mybir  # noqa: E402

F32 = mybir.dt.float32
F32R = mybir.dt.float32r

# Full problem constants
B_FULL, T_FULL, C_FULL = 2, 2048, 2048
N_HEADS, HEAD_DIM = 16, 128
N_CORES = 8
H_LOC = N_HEADS // N_CORES  # 2 heads per core
C_LOC = H_LOC * HEAD_DIM  # 256 output dims per core

WIN = 512  # token window for projections / attention q-window


def build_program(Bb=B_FULL, Tt=T_FULL, Cc=C_FULL):
    """Build the single-core program (SPMD across the 8 cores).

    Per-core DRAM interface:
      xT : [Cc, Bb*Tt]  f32  (x transposed, replicated)
      wq : [Cc, C_LOC]  f32  (Wq rows for this core's heads, transposed,
                              pre-scaled by 1/sqrt(HEAD_DIM))
      wk : [Cc, C_LOC]  f32
      wv : [Cc, C_LOC]  f32
      wo : [C_LOC, Cc]  f32  (Wo columns for this core's heads, transposed)
      y  : [Bb*Tt, Cc]  f32  out (partial sum; host reduces over cores)
    """
    BT = Bb * Tt
    n_kc = Cc // 128  # contraction chunks for projections
    n_win = BT // WIN  # projection token windows
    n_qw = Tt // WIN  # attention q-windows per batch element
    n_bt = BT // 128  # 128-token tiles
    sub = WIN // 128  # 128-token subtiles per window (4)

    nc = bacc.Bacc("TRN2", target_bir_lowering=False, debug=False,
                   num_devices=N_CORES)

    xT_ap = nc.dram_tensor("xT", [Cc, BT], F32R, kind="ExternalInput").ap()
    wq_ap = nc.dram_tensor("wq", [Cc, C_LOC], F32R, kind="ExternalInput").ap()
    wk_ap = nc.dram_tensor("wk", [Cc, C_LOC], F32R, kind="ExternalInput").ap()
    wv_ap = nc.dram_tensor("wv", [Cc, C_LOC], F32R, kind="ExternalInput").ap()
    wo_ap = nc.dram_tensor("wo", [C_LOC, Cc], F32R, kind="ExternalInput").ap()
    y_ap = nc.dram_tensor("y", [BT, Cc], F32, kind="ExternalOutput").ap()

    with tile.TileContext(nc) as tc, ExitStack() as ctx:
        const = ctx.enter_context(tc.tile_pool(name="const", bufs=1))
        wop = ctx.enter_context(tc.tile_pool(name="wop", bufs=1))
        qkv = ctx.enter_context(tc.tile_pool(name="qkv", bufs=1))

        # memset rejects f32r destinations in walrus codegen: set an f32
        # staging tile and convert-copy (bitwise identity) into the f32r one.
        ones_f32 = const.tile([128, 1], F32, tag="ones_f32")
        nc.any.memset(ones_f32[:], 1.0)
        ones_col = const.tile([128, 1], F32R, tag="ones_col")
        nc.vector.tensor_copy(ones_col[:], ones_f32[:])
        ones_rf32 = const.tile([1, 128], F32, tag="ones_rf32")
        nc.any.memset(ones_rf32[:], 1.0)
        ones_row = const.tile([1, 128], F32R, tag="ones_row")
        nc.vector.tensor_copy(ones_row[:], ones_rf32[:])

        # Persistent SBUF tensors
        wo_s = wop.tile([128, H_LOC, Cc], F32R, tag="wo")
        qT_s = qkv.tile([128, H_LOC, BT], F32R, tag="qT")
        kT_s = qkv.tile([128, H_LOC, BT], F32R, tag="kT")
        v_s = qkv.tile([128, n_bt, C_LOC], F32R, tag="v")

        nc.sync.dma_start(wo_s[:], wo_ap.rearrange("(hc p) o -> p hc o", p=128))

        # ---- Stage 1: q/k/v projections --------------------------------
        with nc.named_scope("qkv_proj"), ExitStack() as s1:
            wqkv = s1.enter_context(tc.tile_pool(name="wqkv", bufs=1))
            xpool = s1.enter_context(tc.tile_pool(name="xpool", bufs=3))
            ps_qk = s1.enter_context(
                tc.tile_pool(name="ps_qk", bufs=1, space="PSUM"))
            ps_v = s1.enter_context(
                tc.tile_pool(name="ps_v", bufs=1, space="PSUM"))

            wq_s = wqkv.tile([128, n_kc, C_LOC], F32R, tag="wq")
            wk_s = wqkv.tile([128, n_kc, C_LOC], F32R, tag="wk")
            wv_s = wqkv.tile([128, n_kc, C_LOC], F32R, tag="wv")
            nc.sync.dma_start(wq_s[:],
                              wq_ap.rearrange("(kc p) o -> p kc o", p=128))
            nc.sync.dma_start(wk_s[:],
                              wk_ap.rearrange("(kc p) o -> p kc o", p=128))
            nc.sync.dma_start(wv_s[:],
                              wv_ap.rearrange("(kc p) o -> p kc o", p=128))

            for w in range(n_win):
                toks = slice(w * WIN, (w + 1) * WIN)
                q_ps = [ps_qk.tile([128, WIN], F32, tag=f"q{h}", name=f"q_ps{h}")
                        for h in range(H_LOC)]
                k_ps = [ps_qk.tile([128, WIN], F32, tag=f"k{h}", name=f"k_ps{h}")
                        for h in range(H_LOC)]
                v_ps = [ps_v.tile([128, C_LOC], F32, tag=f"v{j}", name=f"v_ps{j}")
                        for j in range(sub)]
                for kc in range(n_kc):
                    strip = xpool.tile([128, WIN], F32R, tag="strip")
                    nc.sync.dma_start(strip[:],
                                      xT_ap[kc * 128:(kc + 1) * 128, toks])
                    st = (kc == 0)
                    sp = (kc == n_kc - 1)
                    for h in range(H_LOC):
                        hs = slice(h * 128, (h + 1) * 128)
                        nc.tensor.matmul(q_ps[h][:], wq_s[:, kc, hs], strip[:],
                                         start=st, stop=sp)
                        nc.tensor.matmul(k_ps[h][:], wk_s[:, kc, hs], strip[:],
                                         start=st, stop=sp)
                    for j in range(sub):
                        nc.tensor.matmul(v_ps[j][:],
                                         strip[:, j * 128:(j + 1) * 128],
                                         wv_s[:, kc, :], start=st, stop=sp)
                for h in range(H_LOC):
                    nc.scalar.copy(qT_s[:, h, toks], q_ps[h][:])
                    nc.scalar.copy(kT_s[:, h, toks], k_ps[h][:])
                for j in range(sub):
                    nc.vector.tensor_copy(v_s[:, w * sub + j, :], v_ps[j][:])

        # ---- Stages 2+3: attention + output projection, interleaved by
        # batch so y DMA-out of batch 0 overlaps attention of batch 1.
        with nc.named_scope("attention"), ExitStack() as s2:
            ptpool = s2.enter_context(tc.tile_pool(name="ptpool", bufs=3))
            spool = s2.enter_context(tc.tile_pool(name="spool", bufs=2))
            ypool = s2.enter_context(tc.tile_pool(name="ypool", bufs=4))
            ps_at = s2.enter_context(
                tc.tile_pool(name="ps_at", bufs=2, space="PSUM"))

            # attention output (outT layout) reuses qT_s storage: each
            # qT_s[:, h, qsl] slice is last read by this (b,h,qw)'s S^T
            # matmuls, exactly before ot_s[:, h, qsl] is written.
            ot_s = qT_s
            n_nw = Cc // WIN

            for b in range(Bb):
                for h in range(H_LOC):
                    for qw in range(n_qw):
                        qoff = b * Tt + qw * WIN
                        qsl = slice(qoff, qoff + WIN)
                        n_kt = sub * (qw + 1)
                        ot_ps = ps_at.tile([128, WIN], F32, tag="ot")
                        s_ps = ps_at.tile([1, WIN], F32, tag="s")
                        for kt in range(n_kt):
                            koff = b * Tt + kt * 128
                            st_ps = ps_at.tile([128, WIN], F32, tag="st")
                            nc.tensor.matmul(st_ps[:],
                                             kT_s[:, h, koff:koff + 128],
                                             qT_s[:, h, qsl],
                                             start=True, stop=True)
                            pt = ptpool.tile([128, WIN], F32R, tag="pt")
                            nc.scalar.activation(
                                pt[:], st_ps[:],
                                mybir.ActivationFunctionType.Exp)
                            if kt >= qw * sub:
                                # diagonal-overlap tile: zero entries where
                                # global_k > global_q. Predicate keeps where
                                # base - p + f >= 0 (walrus has no is_le).
                                base = qw * WIN - kt * 128
                                nc.gpsimd.affine_select(
                                    out=pt[:], in_=pt[:],
                                    compare_op=mybir.AluOpType.is_ge,
                                    fill=0.0, base=base,
                                    pattern=[[1, WIN]], channel_multiplier=-1,
                                )
                            first = (kt == 0)
                            last = (kt == n_kt - 1)
                            vt = b * (Tt // 128) + kt
                            nc.tensor.matmul(ot_ps[:],
                                             v_s[:, vt, h * 128:(h + 1) * 128],
                                             pt[:], start=first, stop=last)
                            nc.tensor.matmul(s_ps[:], ones_col[:], pt[:],
                                             start=first, stop=last)
                        # normalization: ot_s = ot_ps * bcast(1/s), entirely
                        # off the PE (DVE recip -> gpsimd broadcast -> DVE mul)
                        srec = spool.tile([1, WIN], F32R, tag="srec")
                        with nc.allow_low_precision(reason="f32r == f32 bits"):
                            nc.vector.reciprocal(srec[:], s_ps[:])
                        bc_sb = spool.tile([128, WIN], F32R, tag="bc")
                        nc.gpsimd.partition_broadcast(bc_sb[:], srec[:])
                        nc.scalar.copy(ot_s[:, h, qsl], ot_ps[:])
                        nc.vector.tensor_mul(ot_s[:, h, qsl], ot_s[:, h, qsl],
                                             bc_sb[:])

                # out-projection for this batch's token rows
                with nc.named_scope(f"out_proj{b}"):
                    for bt in range(b * (Tt // 128), (b + 1) * (Tt // 128)):
                        rows = slice(bt * 128, (bt + 1) * 128)
                        for nw in range(n_nw):
                            cols = slice(nw * WIN, (nw + 1) * WIN)
                            y_ps = ps_at.tile([128, WIN], F32, tag="y")
                            for hc in range(H_LOC):
                                nc.tensor.matmul(y_ps[:], ot_s[:, hc, rows],
                                                 wo_s[:, hc, cols],
                                                 start=(hc == 0),
                                                 stop=(hc == H_LOC - 1))
                            y_sb = ypool.tile([128, WIN], F32, tag="ysb")
                            # alternate eviction engine so neither ACT nor
                            # DVE saturates and gates PSUM recycling
                            if (bt * n_nw + nw) % 2 == 0:
                                nc.vector.tensor_copy(y_sb[:], y_ps[:])
                            else:
                                nc.scalar.copy(y_sb[:], y_ps[:])
                            nc.sync.dma_start(y_ap[rows, cols], y_sb[:])

    nc.compile()
    return nc


_PROGRAM = None


def _get_program():
    global _PROGRAM
    if _PROGRAM is None:
        _PROGRAM = build_program()
    return _PROGRAM


def make_in_maps(x, Wq, Wk, Wv, Wo):
    """Host-side sharding: build the per-core input dicts."""
    x = np.asarray(x, dtype=np.float32)
    Wq = np.asarray(Wq, dtype=np.float32)
    Wk = np.asarray(Wk, dtype=np.float32)
    Wv = np.asarray(Wv, dtype=np.float32)
    Wo = np.asarray(Wo, dtype=np.float32)
    BT = x.shape[0] * x.shape[1]
    xT = np.ascontiguousarray(x.reshape(BT, -1).T)
    scale = 1.0 / math.sqrt(HEAD_DIM)
    in_maps = []
    for c in range(N_CORES):
        rows = slice(c * C_LOC, (c + 1) * C_LOC)
        in_maps.append({
            "xT": xT,
            "wq": np.ascontiguousarray(Wq[rows, :].T) * scale,
            "wk": np.ascontiguousarray(Wk[rows, :].T),
            "wv": np.ascontiguousarray(Wv[rows, :].T),
            "wo": np.ascontiguousarray(Wo[:, rows].T),
        })
    return in_maps


def kernel(x, Wq, Wk, Wv, Wo):
    from concourse.bass_utils import run_bass_kernel_spmd

    nc = _get_program()
    in_maps = make_in_maps(x, Wq, Wk, Wv, Wo)
    res = run_bass_kernel_spmd(nc, in_maps, list(range(N_CORES)))
    x = np.asarray(x)
    Bb, Tt, Cc = x.shape
    y = np.zeros((Bb * Tt, Cc), dtype=np.float32)
    for c in range(N_CORES):
        y += res.results[c]["y"]
    return y.reshape(Bb, Tt, Cc)
